# revision 4
# baseline (speedup 1.0000x reference)
"""Trainium2 Bass kernel for nn_Attention_22179211117150 (sparse axial attention).

Strategy (8 NeuronCores, zero collectives):
  - Axial attention: tokens attend within their own frame (N=1024 tokens,
    F=16 frames). 2 frames per core; weights replicated; fully local.
  - Keys/values compressed on host to the kept (mask!=0) positions.
  - All matmuls bf16 (fp32 psum); softmax exp in f32 on ScalarE.
  - Transposed dataflow: qT/kT [d, tokens], simT [keys, queries] with the
    4 quadrants (hr x iw) of one (frame, head-pair, key-tile) in a single
    [128, 2048] psum tile so ONE activation instruction does the exp
    (amortizes the ~352-cycle ACT overhead).
  - Diagonal mask: narrow band multiply on VectorE over a [128, 2, bw]
    strided view covering both heads in one instruction.
  - Softmax denominators ride the av matmul as a ones-column (M=65);
    reciprocal runs directly on the psum denominator row (bf16 out),
    broadcast to 64 partitions via a K=1 matmul, one fused mul normalizes
    and casts to bf16.
  - Emission interleaves projections/out-proj/av at key-tile granularity
    between sim matmuls so the (in-order) PE queue always has fill work
    while ScalarE chews exps, and ScalarE starts ~15us into the kernel.
"""
import numpy as np
import ml_dtypes
from contextlib import ExitStack

import concourse.bass as bass
import concourse.mybir as mybir
import concourse.tile as tile
from concourse import bacc
from concourse.bass_utils import run_bass_kernel_spmd

dt = mybir.dt
AF = mybir.ActivationFunctionType
bf16 = ml_dtypes.bfloat16

B, F, N, H, D, DIM = 1, 16, 1024, 8, 64, 512
NCORES = 8
FPC = F // NCORES          # frames per core
T = FPC * N                # tokens per core
NEG = -1.0e9

TRACE = False
LAST = {}

_nc_cache = {}


def _build(njt, diag, band_lo, band_w):
    nkp = njt * 128
    KV = FPC * nkp                     # kv rows per core (padded)
    nc = bacc.Bacc("TRN2", target_bir_lowering=False, debug=False,
                   num_devices=NCORES)

    xT_d = nc.declare_dram_parameter("xT", [128, 4 * T], dt.bfloat16, isOutput=False)
    xkvT_d = nc.declare_dram_parameter("xkvT", [128, 4 * KV], dt.bfloat16, isOutput=False)
    wq_d = nc.declare_dram_parameter("wq", [128, 4 * 512], dt.bfloat16, isOutput=False)
    wk_d = nc.declare_dram_parameter("wk", [128, 4 * 512], dt.bfloat16, isOutput=False)
    wv_d = nc.declare_dram_parameter("wv", [128, 4 * 512], dt.bfloat16, isOutput=False)
    wo_d = nc.declare_dram_parameter("wo", [128, 4 * 512], dt.bfloat16, isOutput=False)
    eb_d = nc.declare_dram_parameter("eb", [128, njt], dt.float32, isOutput=False)
    if diag:
        mmb_d = nc.declare_dram_parameter("mmb", [128, njt * 2 * band_w],
                                          dt.bfloat16, isOutput=False)
    out_d = nc.declare_dram_parameter("out", [T, DIM], dt.float32, isOutput=True)

    with tile.TileContext(nc) as tc, ExitStack() as ctx:
        consts = ctx.enter_context(tc.tile_pool(name="consts", bufs=1))
        work = ctx.enter_context(tc.tile_pool(name="work", bufs=1))
        etp = ctx.enter_context(tc.tile_pool(name="etp", bufs=12))
        smallp = ctx.enter_context(tc.tile_pool(name="small", bufs=8))
        outp = ctx.enter_context(tc.tile_pool(name="outp", bufs=3))
        dramp = ctx.enter_context(tc.tile_pool(name="dramp", bufs=2, space="DRAM"))
        simp = ctx.enter_context(tc.tile_pool(name="simp", bufs=1, space="PSUM"))
        avp = ctx.enter_context(tc.tile_pool(name="avp", bufs=2, space="PSUM"))
        pp = ctx.enter_context(tc.tile_pool(name="pp", bufs=2, space="PSUM"))

        def load(d, shape, dtype, tag, split=1, eng=None):
            eng = eng or nc.sync
            t = consts.tile(shape, dtype, tag=tag, name=tag)
            n = shape[1]
            step = n // split
            for o in range(0, n, step):
                eng.dma_start(t[:, o:o + step], d[:, o:o + step])
            return t

        wq = load(wq_d, [128, 4 * 512], dt.bfloat16, "wq")
        wk = load(wk_d, [128, 4 * 512], dt.bfloat16, "wk", eng=nc.scalar)
        wv = load(wv_d, [128, 4 * 512], dt.bfloat16, "wv", eng=nc.scalar)
        wo = load(wo_d, [128, 4 * 512], dt.bfloat16, "wo")
        eb = load(eb_d, [128, njt], dt.float32, "eb", eng=nc.scalar)
        if diag:
            mmb = load(mmb_d, [128, njt * 2 * band_w], dt.bfloat16, "mmb",
                       eng=nc.scalar)
        xT = load(xT_d, [128, 4 * T], dt.bfloat16, "xT", split=4)
        xkvT = load(xkvT_d, [128, 4 * KV], dt.bfloat16, "xkvT", split=4,
                    eng=nc.scalar)

        ones_sb = work.tile([128, 64], dt.bfloat16, tag="ones", name="ones")
        nc.vector.memset(ones_sb[:], 1.0)

        # PE warm-up burst while inputs stream in (HAM clock gate).
        warm_src = work.tile([128, 512], dt.bfloat16, tag="warmsrc", name="warmsrc")
        nc.vector.memset(warm_src[:], 0.5)
        wps = pp.tile([128, 512], dt.float32, tag="pp", name="pp_t")
        for wi in range(20):
            nc.tensor.matmul(wps[0:64, :], ones_sb[:, 0:64], warm_src[:],
                             start=(wi == 0), stop=(wi == 19))
        wsb = smallp.tile([1, 64], dt.float32, tag="warm", name="warm_t")
        nc.vector.tensor_copy(wsb[:], wps[0:1, 0:64])
        wdr = dramp.tile([1, 64], dt.float32, tag="wdr", name="wdr_t")
        nc.sync.dma_start(wdr[:], wsb[:])

        qT = [work.tile([128, T], dt.bfloat16, tag=f"qT{hp}", name=f"qT{hp}")
              for hp in range(4)]
        kT = [work.tile([128, KV], dt.bfloat16, tag=f"kT{hp}", name=f"kT{hp}")
              for hp in range(4)]
        vt = [[work.tile([128, 8 * 65], dt.bfloat16, tag=f"v{f}_{jt}",
                         name=f"v{f}_{jt}") for jt in range(njt)]
              for f in range(FPC)]
        aoT = [work.tile([128, T], dt.bfloat16, tag=f"aoT{hp}", name=f"aoT{hp}")
               for hp in range(4)]

        # ---- projection emitters (one (hp, window) slice at a time) ----
        def emit_q_slice(f, hp, iw):
            w0 = f * N + iw * 512
            ps = pp.tile([128, 512], dt.float32, tag="pp", name="pp_t")
            for cc in range(4):
                nc.tensor.matmul(
                    ps[:],
                    wq[:, cc * 512 + hp * 128: cc * 512 + hp * 128 + 128],
                    xT[:, cc * T + w0: cc * T + w0 + 512],
                    start=(cc == 0), stop=(cc == 3))
            nc.vector.tensor_copy(qT[hp][:, w0:w0 + 512], ps[:])

        def emit_k_slice(f, hp, win):
            # windows over nkp cols: (0,512), (512, nkp-512)
            w0, wl = win
            c0 = f * nkp + w0
            ps = pp.tile([128, 512], dt.float32, tag="pp", name="pp_t")
            for cc in range(4):
                nc.tensor.matmul(
                    ps[:, 0:wl],
                    wk[:, cc * 512 + hp * 128: cc * 512 + hp * 128 + 128],
                    xkvT[:, cc * KV + c0: cc * KV + c0 + wl],
                    start=(cc == 0), stop=(cc == 3))
            nc.vector.tensor_copy(kT[hp][:, c0:c0 + wl], ps[:, 0:wl])

        def emit_v_slice(f, jt):
            col0 = f * nkp + jt * 128
            ps = pp.tile([128, 512], dt.float32, tag="pp", name="pp_t")
            for cc in range(4):
                nc.tensor.matmul(ps[:],
                                 xkvT[:, cc * KV + col0: cc * KV + col0 + 128],
                                 wv[:, cc * 512: cc * 512 + 512],
                                 start=(cc == 0), stop=(cc == 3))
            v3 = vt[f][jt][:, :].rearrange("p (h c) -> p h c", c=65)
            p3 = ps[:, :].rearrange("p (h c) -> p h c", c=64)
            nc.vector.tensor_copy(v3[:, :, 0:64], p3[:, :, :])
            nc.vector.memset(v3[:, :, 64:65], 1.0)

        def emit_out_slice(f, tt):
            tg = f * (N // 128) + tt
            ps = pp.tile([128, 512], dt.float32, tag="pp", name="pp_t")
            for hp in range(4):
                nc.tensor.matmul(ps[:],
                                 aoT[hp][:, tg * 128:(tg + 1) * 128],
                                 wo[:, hp * 512:(hp + 1) * 512],
                                 start=(hp == 0), stop=(hp == 3))
            osb = outp.tile([128, 512], dt.float32, tag="osb", name="osb_t")
            nc.scalar.copy(osb[:], ps[:])
            nc.sync.dma_start(out_d[tg * 128:(tg + 1) * 128, :], osb[:])

        # ---- attention pieces ----
        def emit_sim_jt(f, hp, jt):
            """sim quadrants for one key tile -> one [128,2048] act -> ET."""
            st = simp.tile([128, 2048], dt.float32, tag="sim", name="sim_t")
            k0 = f * nkp + jt * 128
            for hr in (0, 1):
                po = 64 * hr
                for iw in (0, 1):
                    nc.tensor.matmul(
                        st[:, hr * 1024 + iw * 512: hr * 1024 + iw * 512 + 512],
                        kT[hp][po:po + 64, k0:k0 + 128],
                        qT[hp][po:po + 64, f * N + iw * 512: f * N + iw * 512 + 512],
                        start=True, stop=True)
            et = etp.tile([128, 2048], dt.bfloat16, tag="et", name="et_t")
            nc.scalar.activation(et[:], st[:], AF.Exp, bias=eb[:, jt:jt + 1])
            if diag:
                lo = band_lo[jt]
                e3 = et[:, :].rearrange("p (h q) -> p h q", h=2)
                m3 = mmb[:, jt * 2 * band_w:(jt + 1) * 2 * band_w].rearrange(
                    "p (h w) -> p h w", h=2)
                nc.vector.tensor_mul(e3[:, :, lo:lo + band_w], e3[:, :, lo:lo + band_w],
                                     m3[:, :, :])
            return et

        def emit_av_combo(f, hp, ET, hr, iw):
            h = hp * 2 + hr
            ps = avp.tile([128, 512], dt.float32, tag="av", name="av_t")
            for jt in range(njt):
                nc.tensor.matmul(
                    ps[0:65, :],
                    vt[f][jt][:, 65 * h: 65 * h + 65],
                    ET[jt][:, hr * 1024 + iw * 512: hr * 1024 + iw * 512 + 512],
                    start=(jt == 0), stop=(jt == njt - 1))
            s_sb = smallp.tile([128, 512], dt.bfloat16, tag="sr", name="sr_t")
            nc.vector.tensor_copy(s_sb[64:65, :], ps[64:65, :])
            psx = pp.tile([128, 512], dt.float32, tag="pp", name="pp_t")
            nc.tensor.matmul(psx[0:64, :], ones_sb[64:65, 0:64], s_sb[64:65, :],
                             start=True, stop=True)
            sr = smallp.tile([64, 512], dt.float32, tag="srec", name="srec_t")
            nc.vector.reciprocal_approx_fast(sr[:], psx[0:64, :])
            win = slice(f * N + iw * 512, f * N + iw * 512 + 512)
            if hr == 0:
                nc.vector.tensor_mul(aoT[hp][0:64, win], ps[0:64, :], sr[:])
            else:
                sc = smallp.tile([64, 512], dt.bfloat16, tag="aosc", name="aosc_t")
                nc.vector.tensor_mul(sc[:], ps[0:64, :], sr[:])
                nc.sync.dma_start(aoT[hp][64:128, win], sc[:])

        # ---- main schedule ----
        # fill units injected between sim key-tiles of the CURRENT group:
        #   av combos of the previous group, projections of the next frame,
        #   out-projection tiles of the finished frame.
        for hp in range(4):
            for iw in range(2):
                emit_q_slice(0, hp, iw)
        kwins = [(0, 512)] + ([(512, nkp - 512)] if nkp > 512 else [])
        for hp in range(4):
            for win in kwins:
                emit_k_slice(0, hp, win)
        for jt in range(njt):
            emit_v_slice(0, jt)

        groups = [(f, hp) for f in range(FPC) for hp in range(4)]
        # fill queue: callables emitted between jt slices
        fills = []
        for g, (f, hp) in enumerate(groups):
            if f == 0:
                # next frame's projections spread over frame-0 groups
                if hp == 0:
                    for xhp in range(4):
                        for iw in range(2):
                            fills.append((g, lambda xhp=xhp, iw=iw:
                                          emit_q_slice(1, xhp, iw)))
                if hp == 1:
                    for xhp in range(4):
                        for win in kwins:
                            fills.append((g, lambda xhp=xhp, win=win:
                                          emit_k_slice(1, xhp, win)))
                if hp == 2:
                    for jt in range(njt):
                        fills.append((g, lambda jt=jt: emit_v_slice(1, jt)))

        fill_idx = 0
        prev = None

        def emit_group(g, f, hp):
            nonlocal fill_idx, prev
            ET = {}
            for jt in range(njt):
                ET[jt] = emit_sim_jt(f, hp, jt)
                # interleave: av combo of prev group on slots 0-3
                if prev is not None and jt < 4:
                    pf, php, pET = prev
                    emit_av_combo(pf, php, pET, jt // 2, jt % 2)
                else:
                    # drain a couple of pending fill units
                    for _ in range(3):
                        if fill_idx < len(fills) and fills[fill_idx][0] <= g:
                            fills[fill_idx][1]()
                            fill_idx += 1
            # after sims of this group: leftover fills for this stage
            while fill_idx < len(fills) and fills[fill_idx][0] <= g:
                fills[fill_idx][1]()
                fill_idx += 1
            if prev is not None:
                pf, php, _ = prev
                if php == 3:
                    for tt in range(N // 128):
                        emit_out_slice(pf, tt)
            prev = (f, hp, ET)

        for g, (f, hp) in enumerate(groups):
            emit_group(g, f, hp)
        # drain last group
        pf, php, pET = prev
        for c in range(4):
            emit_av_combo(pf, php, pET, c // 2, c % 2)
        for tt in range(N // 128):
            emit_out_slice(pf, tt)

    nc.compile()
    return nc


def _chunk_major(a):
    """[512, M] f32 -> [128, 4*M] bf16, contraction chunk-major."""
    m = a.shape[1]
    return np.ascontiguousarray(
        a.reshape(4, 128, m).transpose(1, 0, 2).reshape(128, 4 * m)).astype(bf16)


def kernel(x, W_qkv, W_out, mask, diag):
    x = np.asarray(x, dtype=np.float32).reshape(F * N, DIM)
    W_qkv = np.asarray(W_qkv, dtype=np.float32)
    W_out = np.asarray(W_out, dtype=np.float32)
    maskv = np.asarray(mask).reshape(N)
    diag = int(np.asarray(diag))

    kept = np.flatnonzero(maskv != 0)
    nk = int(kept.size)
    assert nk > 0, "all-masked input not supported"
    njt = (nk + 127) // 128
    nkp = njt * 128

    Wq = W_qkv[:, 0:512] * np.float32(D ** -0.5)
    Wk = W_qkv[:, 512:1024]
    Wv = W_qkv[:, 1024:1536]

    wq_h = _chunk_major(Wq)
    wk_h = _chunk_major(Wk)
    wv_h = _chunk_major(Wv)
    wo_h = _chunk_major(W_out)

    eb_h = np.zeros((128, njt), np.float32)
    for jt in range(njt):
        rows = np.arange(jt * 128, jt * 128 + 128)
        eb_h[:, jt] = np.where(rows < nk, 0.0, NEG)

    if diag:
        los, ws = [], []
        for jt in range(njt):
            idx = kept[jt * 128: min(jt * 128 + 128, nk)]
            lo = int(idx.min()) & ~1
            los.append(lo)
            ws.append(int(idx.max()) + 1 - lo)
        bw = (max(ws) + 1) & ~1
        los = [min(lo, N - bw) for lo in los]
        mmb_h = np.ones((128, njt * 2 * bw), np.float32)
        for jt in range(njt):
            valid = min(128, nk - jt * 128)
            p = np.arange(valid)
            off = kept[jt * 128: jt * 128 + valid] - los[jt]
            mmb_h[p, jt * 2 * bw + off] = 0.0
            mmb_h[p, jt * 2 * bw + bw + off] = 0.0
        mmb_h = mmb_h.astype(bf16)
        band_lo = tuple(los)
    else:
        bw = 0
        band_lo = None
        mmb_h = None

    key = (njt, diag, bw, band_lo)
    if key not in _nc_cache:
        _nc_cache[key] = _build(njt, diag, band_lo, bw)
    nc = _nc_cache[key]

    xbf = x.astype(bf16)
    in_maps = []
    for m in range(NCORES):
        xs = xbf[m * T:(m + 1) * T]                      # [T, DIM] bf16
        xT_h = _chunk_major(np.ascontiguousarray(xs.T.astype(np.float32)))
        kvrows = np.zeros((FPC * nkp, DIM), np.float32)
        for f in range(FPC):
            kvrows[f * nkp: f * nkp + nk] = xs[f * N + kept].astype(np.float32)
        xkvT_h = _chunk_major(np.ascontiguousarray(kvrows.T))
        im = dict(xT=xT_h, xkvT=xkvT_h, wq=wq_h, wk=wk_h, wv=wv_h, wo=wo_h,
                  eb=eb_h)
        if diag:
            im["mmb"] = mmb_h
        in_maps.append(im)

    core_ids = list(range(NCORES))
    if TRACE:
        r = run_bass_kernel_spmd(nc, in_maps, core_ids, trace=True)
        LAST["exec_time_ns"] = r.exec_time_ns
        LAST["results"] = r
        results = r.results
    else:
        results = None
        for attempt in range(3):
            try:
                results = run_bass_kernel_spmd(nc, in_maps, core_ids).results
                break
            except Exception:
                if attempt == 2:
                    raise
                import time as _time
                _time.sleep(2.0)

    out = np.concatenate([np.asarray(results[m]["out"]) for m in range(NCORES)],
                         axis=0)
    return out.reshape(B, F * N, DIM).astype(np.float32)


# revision 6
# speedup vs baseline: 1.1897x; 1.1897x over previous
"""Trainium2 Bass kernel for nn_Attention_22179211117150 (sparse axial attention).

Strategy (8 NeuronCores, zero collectives):
  - Axial attention: tokens attend within their own frame (N=1024 tokens,
    F=16 frames). 2 frames per core; weights replicated; fully local.
  - Keys/values compressed on host to the kept (mask!=0) positions.
  - All matmuls bf16 (fp32 psum); softmax exp in f32 on ScalarE.
  - Transposed dataflow: qT/kT [d, tokens], simT [keys, queries].
  - Per (frame, head-pair, key-tile): two ping-pong [128, 1024] psum sim
    tiles (one per head of the pair), each drained by one [128, 1024] exp
    activation into the shared [128, 2048] ET tile, so the PE<->ACT chain
    double-buffers and semaphore latency hides.
  - Diagonal mask: narrow band multiply on GpSimd over a [128, 2, bw]
    strided view of ET covering both heads in one instruction (keeps the
    loaded VectorE free for psum drains).
  - Softmax denominators ride the av matmul as a ones-column (M=65);
    cast + K=1 broadcast matmul + reciprocal + fused normalize-cast.
  - Demand-driven emission: projection/out-proj/av work is queued as fill
    units and drained between sim key-tiles so the in-order PE queue always
    has work while ScalarE chews exps; ScalarE starts ~15us in.
"""
import numpy as np
import ml_dtypes
from collections import deque
from contextlib import ExitStack

import concourse.bass as bass
import concourse.mybir as mybir
import concourse.tile as tile
from concourse import bacc
from concourse.bass_utils import run_bass_kernel_spmd

dt = mybir.dt
AF = mybir.ActivationFunctionType
bf16 = ml_dtypes.bfloat16

B, F, N, H, D, DIM = 1, 16, 1024, 8, 64, 512
NCORES = 8
FPC = F // NCORES          # frames per core
T = FPC * N                # tokens per core
NEG = -1.0e9

BAND_ON_GPSIMD = True

TRACE = False
LAST = {}

_nc_cache = {}


def _build(njt, diag, band_lo, band_w):
    nkp = njt * 128
    KV = FPC * nkp                     # kv rows per core (padded)
    nc = bacc.Bacc("TRN2", target_bir_lowering=False, debug=False,
                   num_devices=NCORES)

    xT_d = nc.declare_dram_parameter("xT", [128, 4 * T], dt.bfloat16, isOutput=False)
    xkvT_d = nc.declare_dram_parameter("xkvT", [128, 4 * KV], dt.bfloat16, isOutput=False)
    wq_d = nc.declare_dram_parameter("wq", [128, 4 * 512], dt.bfloat16, isOutput=False)
    wk_d = nc.declare_dram_parameter("wk", [128, 4 * 512], dt.bfloat16, isOutput=False)
    wv_d = nc.declare_dram_parameter("wv", [128, 4 * 512], dt.bfloat16, isOutput=False)
    wo_d = nc.declare_dram_parameter("wo", [128, 4 * 512], dt.bfloat16, isOutput=False)
    eb_d = nc.declare_dram_parameter("eb", [128, njt], dt.float32, isOutput=False)
    if diag:
        mmb_d = nc.declare_dram_parameter("mmb", [128, njt * 2 * band_w],
                                          dt.bfloat16, isOutput=False)
    out_d = nc.declare_dram_parameter("out", [T, DIM], dt.float32, isOutput=True)

    with tile.TileContext(nc) as tc, ExitStack() as ctx:
        consts = ctx.enter_context(tc.tile_pool(name="consts", bufs=1))
        work = ctx.enter_context(tc.tile_pool(name="work", bufs=1))
        etp = ctx.enter_context(tc.tile_pool(name="etp", bufs=12))
        smallp = ctx.enter_context(tc.tile_pool(name="small", bufs=6))
        outp = ctx.enter_context(tc.tile_pool(name="outp", bufs=3))
        dramp = ctx.enter_context(tc.tile_pool(name="dramp", bufs=2, space="DRAM"))
        simp = ctx.enter_context(tc.tile_pool(name="simp", bufs=2, space="PSUM"))
        avp = ctx.enter_context(tc.tile_pool(name="avp", bufs=2, space="PSUM"))
        pp = ctx.enter_context(tc.tile_pool(name="pp", bufs=2, space="PSUM"))

        def load(d, shape, dtype, tag, eng=None, frame_split=False):
            eng = eng or nc.sync
            t = consts.tile(shape, dtype, tag=tag, name=tag)
            n = shape[1]
            if frame_split:
                # 4 contraction chunks x FPC frame-halves; frame-0 halves
                # first so frame-0 projections start as early as possible.
                half = n // 8
                for fh in range(2):
                    for cc in range(4):
                        o = cc * (n // 4) + fh * half
                        eng.dma_start(t[:, o:o + half], d[:, o:o + half])
            else:
                eng.dma_start(t[:], d[:])
            return t

        wq = load(wq_d, [128, 4 * 512], dt.bfloat16, "wq")
        wk = load(wk_d, [128, 4 * 512], dt.bfloat16, "wk", eng=nc.scalar)
        eb = load(eb_d, [128, njt], dt.float32, "eb", eng=nc.scalar)
        xT = load(xT_d, [128, 4 * T], dt.bfloat16, "xT", frame_split=True)
        xkvT = load(xkvT_d, [128, 4 * KV], dt.bfloat16, "xkvT", eng=nc.scalar,
                    frame_split=True)
        wv = load(wv_d, [128, 4 * 512], dt.bfloat16, "wv", eng=nc.scalar)
        wo = load(wo_d, [128, 4 * 512], dt.bfloat16, "wo")
        if diag:
            mmb = load(mmb_d, [128, njt * 2 * band_w], dt.bfloat16, "mmb",
                       eng=nc.gpsimd)

        ones_sb = work.tile([128, 64], dt.bfloat16, tag="ones", name="ones")
        nc.vector.memset(ones_sb[:], 1.0)

        # PE warm-up burst while inputs stream in (HAM clock gate).
        warm_src = work.tile([128, 512], dt.bfloat16, tag="warmsrc", name="warmsrc")
        nc.vector.memset(warm_src[:], 0.5)
        wps = pp.tile([128, 512], dt.float32, tag="pp", name="pp_t")
        for wi in range(20):
            nc.tensor.matmul(wps[0:64, :], ones_sb[:, 0:64], warm_src[:],
                             start=(wi == 0), stop=(wi == 19))
        wsb = smallp.tile([1, 64], dt.float32, tag="warm", name="warm_t")
        nc.vector.tensor_copy(wsb[:], wps[0:1, 0:64])
        wdr = dramp.tile([1, 64], dt.float32, tag="wdr", name="wdr_t")
        nc.sync.dma_start(wdr[:], wsb[:])

        qT = [work.tile([128, T], dt.bfloat16, tag=f"qT{hp}", name=f"qT{hp}")
              for hp in range(4)]
        kT = [work.tile([128, KV], dt.bfloat16, tag=f"kT{hp}", name=f"kT{hp}")
              for hp in range(4)]
        vt = [[work.tile([128, 8 * 65], dt.bfloat16, tag=f"v{f}_{jt}",
                         name=f"v{f}_{jt}") for jt in range(njt)]
              for f in range(FPC)]
        aoT = [work.tile([128, T], dt.bfloat16, tag=f"aoT{hp}", name=f"aoT{hp}")
               for hp in range(4)]

        kwins = [(0, 512)] + ([(512, nkp - 512)] if nkp > 512 else [])

        # ---- emitters ----
        def emit_q_slice(f, hp, iw):
            w0 = f * N + iw * 512
            ps = pp.tile([128, 512], dt.float32, tag="pp", name="pp_t")
            for cc in range(4):
                nc.tensor.matmul(
                    ps[:],
                    wq[:, cc * 512 + hp * 128: cc * 512 + hp * 128 + 128],
                    xT[:, cc * T + w0: cc * T + w0 + 512],
                    start=(cc == 0), stop=(cc == 3))
            nc.vector.tensor_copy(qT[hp][:, w0:w0 + 512], ps[:])

        def emit_k_slice(f, hp, win):
            w0, wl = win
            c0 = f * nkp + w0
            ps = pp.tile([128, 512], dt.float32, tag="pp", name="pp_t")
            for cc in range(4):
                nc.tensor.matmul(
                    ps[:, 0:wl],
                    wk[:, cc * 512 + hp * 128: cc * 512 + hp * 128 + 128],
                    xkvT[:, cc * KV + c0: cc * KV + c0 + wl],
                    start=(cc == 0), stop=(cc == 3))
            nc.vector.tensor_copy(kT[hp][:, c0:c0 + wl], ps[:, 0:wl])

        def emit_v_slice(f, jt):
            col0 = f * nkp + jt * 128
            ps = pp.tile([128, 512], dt.float32, tag="pp", name="pp_t")
            for cc in range(4):
                nc.tensor.matmul(ps[:],
                                 xkvT[:, cc * KV + col0: cc * KV + col0 + 128],
                                 wv[:, cc * 512: cc * 512 + 512],
                                 start=(cc == 0), stop=(cc == 3))
            v3 = vt[f][jt][:, :].rearrange("p (h c) -> p h c", c=65)
            p3 = ps[:, :].rearrange("p (h c) -> p h c", c=64)
            nc.vector.tensor_copy(v3[:, :, 0:64], p3[:, :, :])
            nc.vector.memset(v3[:, :, 64:65], 1.0)

        def emit_out_slice(f, tt):
            tg = f * (N // 128) + tt
            ps = pp.tile([128, 512], dt.float32, tag="pp", name="pp_t")
            for hp in range(4):
                nc.tensor.matmul(ps[:],
                                 aoT[hp][:, tg * 128:(tg + 1) * 128],
                                 wo[:, hp * 512:(hp + 1) * 512],
                                 start=(hp == 0), stop=(hp == 3))
            osb = outp.tile([128, 512], dt.float32, tag="osb", name="osb_t")
            nc.scalar.copy(osb[:], ps[:])
            nc.sync.dma_start(out_d[tg * 128:(tg + 1) * 128, :], osb[:])

        def emit_av_combo(f, hp, ET, hr, iw):
            h = hp * 2 + hr
            ps = avp.tile([128, 512], dt.float32, tag="av", name="av_t")
            for jt in range(njt):
                nc.tensor.matmul(
                    ps[0:65, :],
                    vt[f][jt][:, 65 * h: 65 * h + 65],
                    ET[jt][:, hr * 1024 + iw * 512: hr * 1024 + iw * 512 + 512],
                    start=(jt == 0), stop=(jt == njt - 1))
            s_sb = smallp.tile([128, 512], dt.bfloat16, tag="sr", name="sr_t")
            nc.vector.tensor_copy(s_sb[64:65, :], ps[64:65, :])
            psx = pp.tile([128, 512], dt.float32, tag="pp", name="pp_t")
            nc.tensor.matmul(psx[0:64, :], ones_sb[64:65, 0:64], s_sb[64:65, :],
                             start=True, stop=True)
            sr = smallp.tile([64, 512], dt.float32, tag="srec", name="srec_t")
            nc.vector.reciprocal_approx_fast(sr[:], psx[0:64, :])
            win = slice(f * N + iw * 512, f * N + iw * 512 + 512)
            if hr == 0:
                nc.vector.tensor_mul(aoT[hp][0:64, win], ps[0:64, :], sr[:])
            else:
                sc = smallp.tile([64, 512], dt.bfloat16, tag="aosc", name="aosc_t")
                nc.vector.tensor_mul(sc[:], ps[0:64, :], sr[:])
                nc.sync.dma_start(aoT[hp][64:128, win], sc[:])

        def emit_sim_hr(f, hp, jt, hr, et):
            st = simp.tile([128, 1024], dt.float32, tag="sim", name="sim_t")
            k0 = f * nkp + jt * 128
            po = 64 * hr
            for iw in (0, 1):
                nc.tensor.matmul(
                    st[:, iw * 512: iw * 512 + 512],
                    kT[hp][po:po + 64, k0:k0 + 128],
                    qT[hp][po:po + 64, f * N + iw * 512: f * N + iw * 512 + 512],
                    start=True, stop=True)
            nc.scalar.activation(et[:, hr * 1024:(hr + 1) * 1024], st[:],
                                 AF.Exp, bias=eb[:, jt:jt + 1])

        def emit_band(jt, et):
            lo = band_lo[jt]
            e3 = et[:, :].rearrange("p (h q) -> p h q", h=2)
            m3 = mmb[:, jt * 2 * band_w:(jt + 1) * 2 * band_w].rearrange(
                "p (h w) -> p h w", h=2)
            eng = nc.gpsimd if BAND_ON_GPSIMD else nc.vector
            eng.tensor_mul(e3[:, :, lo:lo + band_w], e3[:, :, lo:lo + band_w],
                           m3[:, :, :])

        # ---- demand-driven schedule ----
        fills = deque()
        for hp in range(4):
            for iw in range(2):
                fills.append(('q', 0, hp, iw))
        for hp in range(4):
            for wi, win in enumerate(kwins):
                fills.append(('k', 0, hp, wi))
        for jt in range(njt):
            fills.append(('v', 0, jt))
        for hp in range(4):
            for iw in range(2):
                fills.append(('q', 1, hp, iw))
        for hp in range(4):
            for wi, win in enumerate(kwins):
                fills.append(('k', 1, hp, wi))
        for jt in range(njt):
            fills.append(('v', 1, jt))

        def run_fill(u):
            kind = u[0]
            if kind == 'q':
                emit_q_slice(u[1], u[2], u[3])
            elif kind == 'k':
                emit_k_slice(u[1], u[2], kwins[u[3]])
            elif kind == 'v':
                emit_v_slice(u[1], u[2])
            elif kind == 'o':
                emit_out_slice(u[1], u[2])

        def drain_matching(pred):
            rest = deque()
            while fills:
                u = fills.popleft()
                if pred(u):
                    run_fill(u)
                else:
                    rest.append(u)
            fills.extend(rest)

        def drain_some(k):
            for _ in range(k):
                if fills:
                    run_fill(fills.popleft())

        prev = None
        groups = [(f, hp) for f in range(FPC) for hp in range(4)]
        for g, (f, hp) in enumerate(groups):
            # prerequisites: q/k slices of this group's (f, hp)
            drain_matching(lambda u: u[0] in ('q', 'k') and u[1] == f
                           and u[2] == hp)
            if prev is not None:
                # av of prev group needs its frame's v tiles
                drain_matching(lambda u: u[0] == 'v' and u[1] == prev[0])
            ET = {jt: etp.tile([128, 2048], dt.bfloat16, tag="et", name="et_t")
                  for jt in range(njt)}
            for jt in range(njt):
                emit_sim_hr(f, hp, jt, 0, ET[jt])
                if prev is not None and jt < 4:
                    pf, php, pET = prev
                    emit_av_combo(pf, php, pET, jt // 2, jt % 2)
                else:
                    drain_some(2)
                emit_sim_hr(f, hp, jt, 1, ET[jt])
                if diag:
                    emit_band(jt, ET[jt])
                drain_some(1)
            if prev is not None and prev[1] == 3:
                for tt in range(N // 128):
                    fills.append(('o', prev[0], tt))
            prev = (f, hp, ET)

        pf, php, pET = prev
        for c in range(4):
            emit_av_combo(pf, php, pET, c // 2, c % 2)
            drain_some(2)
        while fills:
            run_fill(fills.popleft())
        for tt in range(N // 128):
            emit_out_slice(pf, tt)

    nc.compile()
    return nc


def _chunk_major(a):
    """[512, M] f32 -> [128, 4*M] bf16, contraction chunk-major."""
    m = a.shape[1]
    return np.ascontiguousarray(
        a.reshape(4, 128, m).transpose(1, 0, 2).reshape(128, 4 * m)).astype(bf16)


def kernel(x, W_qkv, W_out, mask, diag):
    x = np.asarray(x, dtype=np.float32).reshape(F * N, DIM)
    W_qkv = np.asarray(W_qkv, dtype=np.float32)
    W_out = np.asarray(W_out, dtype=np.float32)
    maskv = np.asarray(mask).reshape(N)
    diag = int(np.asarray(diag))

    kept = np.flatnonzero(maskv != 0)
    nk = int(kept.size)
    assert nk > 0, "all-masked input not supported"
    njt = (nk + 127) // 128
    nkp = njt * 128

    Wq = W_qkv[:, 0:512] * np.float32(D ** -0.5)
    Wk = W_qkv[:, 512:1024]
    Wv = W_qkv[:, 1024:1536]

    wq_h = _chunk_major(Wq)
    wk_h = _chunk_major(Wk)
    wv_h = _chunk_major(Wv)
    wo_h = _chunk_major(W_out)

    eb_h = np.zeros((128, njt), np.float32)
    for jt in range(njt):
        rows = np.arange(jt * 128, jt * 128 + 128)
        eb_h[:, jt] = np.where(rows < nk, 0.0, NEG)

    if diag:
        los, ws = [], []
        for jt in range(njt):
            idx = kept[jt * 128: min(jt * 128 + 128, nk)]
            lo = int(idx.min()) & ~1
            los.append(lo)
            ws.append(int(idx.max()) + 1 - lo)
        bw = (max(ws) + 1) & ~1
        los = [min(lo, N - bw) for lo in los]
        mmb_h = np.ones((128, njt * 2 * bw), np.float32)
        for jt in range(njt):
            valid = min(128, nk - jt * 128)
            p = np.arange(valid)
            off = kept[jt * 128: jt * 128 + valid] - los[jt]
            mmb_h[p, jt * 2 * bw + off] = 0.0
            mmb_h[p, jt * 2 * bw + bw + off] = 0.0
        mmb_h = mmb_h.astype(bf16)
        band_lo = tuple(los)
    else:
        bw = 0
        band_lo = None
        mmb_h = None

    key = (njt, diag, bw, band_lo)
    if key not in _nc_cache:
        _nc_cache[key] = _build(njt, diag, band_lo, bw)
    nc = _nc_cache[key]

    xbf = x.astype(bf16)
    in_maps = []
    for m in range(NCORES):
        xs = xbf[m * T:(m + 1) * T]                      # [T, DIM] bf16
        xT_h = _chunk_major(np.ascontiguousarray(xs.T.astype(np.float32)))
        kvrows = np.zeros((FPC * nkp, DIM), np.float32)
        for f in range(FPC):
            kvrows[f * nkp: f * nkp + nk] = xs[f * N + kept].astype(np.float32)
        xkvT_h = _chunk_major(np.ascontiguousarray(kvrows.T))
        im = dict(xT=xT_h, xkvT=xkvT_h, wq=wq_h, wk=wk_h, wv=wv_h, wo=wo_h,
                  eb=eb_h)
        if diag:
            im["mmb"] = mmb_h
        in_maps.append(im)

    core_ids = list(range(NCORES))
    if TRACE:
        r = run_bass_kernel_spmd(nc, in_maps, core_ids, trace=True)
        LAST["exec_time_ns"] = r.exec_time_ns
        LAST["results"] = r
        results = r.results
    else:
        results = None
        for attempt in range(3):
            try:
                results = run_bass_kernel_spmd(nc, in_maps, core_ids).results
                break
            except Exception:
                if attempt == 2:
                    raise
                import time as _time
                _time.sleep(2.0)

    out = np.concatenate([np.asarray(results[m]["out"]) for m in range(NCORES)],
                         axis=0)
    return out.reshape(B, F * N, DIM).astype(np.float32)
